# revision 15
# baseline (speedup 1.0000x reference)
"""Trainium2 Bass kernel for nn_LocalDownsample (segment mean-pool via one-hot matmul).

Contract: kernel(**inputs) takes FULL inputs (x [8,4096,512] f32,
regions [8,4096] i64, max_n=512), returns FULL output [8,512,512] f32.

Sharding: pure data parallel — batch b -> core b. Per core:
  out[n-1, :] = mean over tokens t with regions[t] == n of x[t, :]   (0 if empty)

Device algorithm per core (T=4096 tokens, C=512 channels, N=512 regions):
  tokens laid out as t = p*32 + j (p = SBUF partition, j = k-tile).
  Phase A (overlaps the ramped x DMA stream): build 32 one-hot tiles
    oh_j [128,512] fp16 = (iota == regions[p,j]) on DVE (all-fp16, 2x mode),
    accumulate oh_sum += oh_j on DVE; counts via one matmul
    cnt[1,512] = ones[128,1].T @ oh_sum; recip = 1/max(cnt,1), PE-transposed
    to rt [128,4] in the DMA shadow.
  Phase B: acc[m][128,512] fp32 PSUM += oh_j[:, mP:(m+1)P].T @ fp16(x)_j
    (+ residual matmul with fp16(x - fp16(x)) when split=True -> ~1e-7 rel err;
     without it ~2.5e-4). Final chunk runs m-major so acc banks close early.
  Phase C (per m, pipelined): osb_m = acc[m] * rt[:, m] on DVE, 256 KiB DMA out.
"""

import numpy as np

import concourse.bacc as bacc
import concourse.bass as bass  # noqa: F401
import concourse.mybir as mybir
import concourse.tile as tile
from concourse.bass_utils import run_bass_kernel_spmd

P = 128          # SBUF partitions
T = 4096         # tokens per batch
C = 512          # channels
NR = 512         # number of regions (max_n)
JT = T // P      # 32 k-tiles
MC = NR // P     # 4 output row chunks
NCORES = 8
CHUNKS = (1, 1, 2, 4, 8, 8, 8)   # k-tiles per x DMA chunk (ramped start)

F16 = mybir.dt.float16
F32 = mybir.dt.float32
I32 = mybir.dt.int32

DEFAULT_CFG = dict(split=True, repeats=1)

_CACHE = {}


def _build(split=True, repeats=1):
    assert sum(CHUNKS) == JT
    nc = bacc.Bacc(None, target_bir_lowering=False)
    x_d = nc.dram_tensor("x", [T, C], F32, kind="ExternalInput")
    r_d = nc.dram_tensor("regions", [T], I32, kind="ExternalInput")
    o_d = nc.dram_tensor("out", [NR, C], F32, kind="ExternalOutput")

    with tile.TileContext(nc) as tc:
        with (
            tc.tile_pool(name="const", bufs=1) as cpool,
            tc.tile_pool(name="xf", bufs=len(CHUNKS)) as xf_pool,
            tc.tile_pool(name="x16", bufs=10) as x16_pool,
            tc.tile_pool(name="oh", bufs=JT) as oh_pool,
            tc.tile_pool(name="eplg", bufs=1) as out_pool,
            tc.tile_pool(name="psum", bufs=1, space="PSUM") as psum_pool,
        ):
            # --- constants; regions ride the Activation HWDGE ring so the
            # SP ring belongs to the x stream from t=0 ---
            r_i = cpool.tile([P, JT], I32, tag="r_i")
            nc.scalar.dma_start(r_i[:], r_d.rearrange("(p j) -> p j", p=P))
            r_f = cpool.tile([P, JT], F32, tag="r_f")
            nc.vector.tensor_copy(r_f[:], r_i[:])

            iota16 = cpool.tile([P, NR], F16, tag="iota16")
            nc.gpsimd.iota(
                iota16[:], pattern=[[1, NR]], base=1, channel_multiplier=0,
                allow_small_or_imprecise_dtypes=True,  # 1..512 exact in fp16
            )

            ones_st = cpool.tile([P, 1], F32, tag="ones_st")
            nc.vector.memset(ones_st[:], 1.0)
            ident1 = cpool.tile([1, 1], F32, tag="ident1")
            nc.vector.memset(ident1[:], 1.0)

            def body():
                # x stream: queue all chunk DMAs up front (FIFO on the SP ring,
                # ramped sizes so the first matmuls can start early)
                xv = x_d.rearrange("(p j) c -> p j c", p=P)
                xf = []          # per j: (chunk_tile, index within chunk)
                j0 = 0
                for ci, csz in enumerate(CHUNKS):
                    t = xf_pool.tile([P, csz, C], F32, name=f"xfc{ci}", tag="xf")
                    nc.sync.dma_start(t[:], xv[:, j0 : j0 + csz, :])
                    for jj in range(csz):
                        xf.append((t, jj))
                    j0 += csz

                # one PSUM bank per accumulation group: start=True clears
                # has_written for the whole bank
                acc = [
                    psum_pool.tile([P, C], F32, name=f"acc{m}", tag=f"acc{m}")
                    for m in range(MC)
                ]
                cnt = psum_pool.tile([1, NR], F32, tag="cnt")

                # --- Phases A+B interleaved: per k-tile, DVE builds the
                # one-hot and running sum (and the fp16 residual of x when
                # split) while PE streams the main matmuls ---
                oh = []
                # fp32 so counts stay exact even if one region owned all
                # 4096 tokens (fp16 integers are only exact to 2048)
                oh_sum = out_pool.tile([P, NR], F32, tag="oh_sum")
                x16s = {}

                def load_x16(j):
                    xt, jj = xf[j]
                    x16 = x16_pool.tile([P, C], F16, name=f"x16_{j}", tag="x16")
                    if j < 2:
                        nc.vector.tensor_copy(x16[:], xt[:, jj, :])
                    else:
                        nc.scalar.copy(x16[:], xt[:, jj, :])
                    xlo = None
                    if split:
                        xlo = x16_pool.tile([P, C], F16, name=f"xlo_{j}", tag="xlo")
                        nc.vector.tensor_tensor(
                            out=xlo[:], in0=xt[:, jj, :], in1=x16[:],
                            op=mybir.AluOpType.subtract,
                        )
                    x16s[j] = (x16, xlo)

                def mm(m, j):
                    x16, xlo = x16s[j]
                    nc.tensor.matmul(
                        acc[m][:],
                        lhsT=oh[j][:, m * P : (m + 1) * P],
                        rhs=x16[:],
                        start=(j == 0),
                        stop=(j == JT - 1) and not split,
                        skip_group_check=True,
                    )
                    if split:
                        nc.tensor.matmul(
                            acc[m][:],
                            lhsT=oh[j][:, m * P : (m + 1) * P],
                            rhs=xlo[:],
                            start=False,
                            stop=(j == JT - 1),
                            skip_group_check=True,
                        )

                LAST = JT - CHUNKS[-1]     # final chunk runs m-major
                for j in range(JT):
                    t = oh_pool.tile([P, NR], F16, name=f"oh{j}", tag="oh")
                    nc.vector.tensor_scalar(
                        out=t[:],
                        in0=iota16[:],
                        scalar1=r_f[:, j : j + 1],
                        scalar2=None,
                        op0=mybir.AluOpType.is_equal,
                    )
                    oh.append(t)
                    load_x16(j)
                    if j == 0:
                        nc.vector.tensor_copy(oh_sum[:], t[:])
                    else:
                        nc.vector.tensor_tensor(
                            out=oh_sum[:], in0=oh_sum[:], in1=t[:],
                            op=mybir.AluOpType.add,
                        )
                    if j < LAST:
                        for m in range(MC):
                            mm(m, j)

                # counts + reciprocal + PE transpose to [128, 4] — emitted
                # late so the PE stream isn't blocked on the full oh_sum chain
                nc.tensor.matmul(
                    cnt[:], lhsT=ones_st[:], rhs=oh_sum[:],
                    start=True, stop=True, skip_group_check=True,
                )
                csb = out_pool.tile([1, NR], F32, tag="csb")
                nc.vector.tensor_scalar_max(csb[:], cnt[:], 1.0)
                recip = out_pool.tile([1, NR], F32, tag="recip")
                nc.vector.reciprocal(recip[:], csb[:])
                rt = out_pool.tile([P, MC], F32, tag="rt")
                for m in range(MC):
                    rp = psum_pool.tile([P, 1], F32, name=f"rp{m}", tag=f"rp{m % 2}")
                    nc.tensor.transpose(
                        rp[:], recip[:, m * P : (m + 1) * P], ident1[:]
                    )
                    nc.vector.tensor_copy(rt[:, m : m + 1], rp[:])

                osb = out_pool.tile([P, MC, C], F32, tag="osb")
                for m in range(MC):
                    for j in range(LAST, JT):
                        mm(m, j)
                    # --- Phase C (per m, overlaps later m's matmuls) ---
                    nc.vector.tensor_scalar(
                        out=osb[:, m, :],
                        in0=acc[m][:],
                        scalar1=rt[:, m : m + 1],
                        scalar2=None,
                        op0=mybir.AluOpType.mult,
                    )
                    nc.sync.dma_start(o_d[m * P : (m + 1) * P, :], osb[:, m, :])

            if repeats == 1:
                body()
            else:
                with tc.For_i(0, repeats, 1, hint_engines=(mybir.EngineType.PE,)):
                    body()

    nc.compile()
    return nc


def _get_nc(**cfg):
    cfg = {**DEFAULT_CFG, **cfg}
    key = tuple(sorted(cfg.items()))
    if key not in _CACHE:
        _CACHE[key] = _build(**cfg)
    return _CACHE[key]


def kernel(x, regions, max_n, _trace=False, _tmpdir=None, _cfg=None):
    x = np.asarray(x, dtype=np.float32)
    regions = np.asarray(regions)
    assert x.shape == (NCORES, T, C), x.shape
    assert regions.shape == (NCORES, T), regions.shape
    assert int(np.asarray(max_n)) == NR

    r32 = np.ascontiguousarray(regions.astype(np.int32))

    nc = _get_nc(**(_cfg or {}))
    in_maps = [
        {"x": np.ascontiguousarray(x[b]), "regions": r32[b]} for b in range(NCORES)
    ]
    res = run_bass_kernel_spmd(
        nc,
        in_maps,
        core_ids=list(range(NCORES)),
        trace=_trace,
        tmpdir=_tmpdir,
    )
    out = np.stack([res.results[b]["out"] for b in range(NCORES)], axis=0)
    if _trace:
        kernel._last_results = res
    return out


# revision 17
# speedup vs baseline: 1.0114x; 1.0114x over previous
"""Trainium2 Bass kernel for nn_LocalDownsample (segment mean-pool via one-hot matmul).

Contract: kernel(**inputs) takes FULL inputs (x [8,4096,512] f32,
regions [8,4096] i64, max_n=512), returns FULL output [8,512,512] f32.

Sharding: pure data parallel — batch b -> core b. Per core:
  out[n-1, :] = mean over tokens t with regions[t] == n of x[t, :]   (0 if empty)

Device algorithm per core (T=4096 tokens, C=512 channels, N=512 regions):
  tokens laid out as t = p*32 + j (p = SBUF partition, j = k-tile).
  Phase A (overlaps the ramped x DMA stream): build 32 one-hot tiles
    oh_j [128,512] fp16 = (iota == regions[p,j]) on DVE (all-fp16, 2x mode),
    accumulate oh_sum += oh_j on DVE; counts via one matmul
    cnt[1,512] = ones[128,1].T @ oh_sum; recip = 1/max(cnt,1), PE-transposed
    to rt [128,4] in the DMA shadow.
  Phase B: acc[m][128,512] fp32 PSUM += oh_j[:, mP:(m+1)P].T @ fp16(x)_j
    (+ residual matmul with fp16(x - fp16(x)) when split=True -> ~1e-7 rel err;
     without it ~2.5e-4). Final chunk runs m-major so acc banks close early.
  Phase C (per m, pipelined): osb_m = acc[m] * rt[:, m] on DVE, 256 KiB DMA out.
"""

import numpy as np

import concourse.bacc as bacc
import concourse.bass as bass  # noqa: F401
import concourse.mybir as mybir
import concourse.tile as tile
from concourse.bass_utils import run_bass_kernel_spmd

P = 128          # SBUF partitions
T = 4096         # tokens per batch
C = 512          # channels
NR = 512         # number of regions (max_n)
JT = T // P      # 32 k-tiles
MC = NR // P     # 4 output row chunks
NCORES = 8
CHUNKS = (1, 1, 2, 4, 8, 8, 8)   # k-tiles per x DMA chunk (ramped start)

F16 = mybir.dt.float16
F32 = mybir.dt.float32
I32 = mybir.dt.int32

DEFAULT_CFG = dict(split=True, repeats=1)

_CACHE = {}


def _build(split=True, repeats=1):
    assert sum(CHUNKS) == JT
    nc = bacc.Bacc(None, target_bir_lowering=False)
    x_d = nc.dram_tensor("x", [T, C], F32, kind="ExternalInput")
    r_d = nc.dram_tensor("regions", [T], I32, kind="ExternalInput")
    o_d = nc.dram_tensor("out", [NR, C], F32, kind="ExternalOutput")

    with tile.TileContext(nc) as tc:
        with (
            tc.tile_pool(name="const", bufs=1) as cpool,
            tc.tile_pool(name="xf", bufs=len(CHUNKS)) as xf_pool,
            tc.tile_pool(name="x16", bufs=10) as x16_pool,
            tc.tile_pool(name="oh", bufs=JT) as oh_pool,
            tc.tile_pool(name="eplg", bufs=1) as out_pool,
            tc.tile_pool(name="psum", bufs=1, space="PSUM") as psum_pool,
        ):
            # --- constants; regions ride the Activation HWDGE ring so the
            # SP ring belongs to the x stream from t=0 ---
            r_i = cpool.tile([P, JT], I32, tag="r_i")
            nc.scalar.dma_start(r_i[:], r_d.rearrange("(p j) -> p j", p=P))
            r_f = cpool.tile([P, JT], F32, tag="r_f")
            nc.vector.tensor_copy(r_f[:], r_i[:])

            iota16 = cpool.tile([P, NR], F16, tag="iota16")
            nc.gpsimd.iota(
                iota16[:], pattern=[[1, NR]], base=1, channel_multiplier=0,
                allow_small_or_imprecise_dtypes=True,  # 1..512 exact in fp16
            )

            ones_st = cpool.tile([P, 1], F32, tag="ones_st")
            nc.vector.memset(ones_st[:], 1.0)
            ident1 = cpool.tile([1, 1], F32, tag="ident1")
            nc.vector.memset(ident1[:], 1.0)

            def body():
                # x stream: queue all chunk DMAs up front (FIFO on the SP ring,
                # ramped sizes so the first matmuls can start early)
                xv = x_d.rearrange("(p j) c -> p j c", p=P)
                xf = []          # per j: (chunk_tile, index within chunk)
                j0 = 0
                for ci, csz in enumerate(CHUNKS):
                    t = xf_pool.tile([P, csz, C], F32, name=f"xfc{ci}", tag="xf")
                    nc.sync.dma_start(t[:], xv[:, j0 : j0 + csz, :])
                    for jj in range(csz):
                        xf.append((t, jj))
                    j0 += csz

                # one PSUM bank per accumulation group: start=True clears
                # has_written for the whole bank
                acc = [
                    psum_pool.tile([P, C], F32, name=f"acc{m}", tag=f"acc{m}")
                    for m in range(MC)
                ]
                cnt = psum_pool.tile([1, NR], F32, tag="cnt")

                # --- Phases A+B interleaved: per k-tile, DVE builds the
                # one-hot and running sum (and the fp16 residual of x when
                # split) while PE streams the main matmuls ---
                oh = []
                # fp32 so counts stay exact even if one region owned all
                # 4096 tokens (fp16 integers are only exact to 2048)
                oh_sum = out_pool.tile([P, NR], F32, tag="oh_sum")
                x16s = {}

                def load_x16(j):
                    xt, jj = xf[j]
                    x16 = x16_pool.tile([P, C], F16, name=f"x16_{j}", tag="x16")
                    if j < 2:
                        nc.vector.tensor_copy(x16[:], xt[:, jj, :])
                    else:
                        nc.scalar.copy(x16[:], xt[:, jj, :])
                    xlo = None
                    if split:
                        xlo = x16_pool.tile([P, C], F16, name=f"xlo_{j}", tag="xlo")
                        nc.vector.tensor_tensor(
                            out=xlo[:], in0=xt[:, jj, :], in1=x16[:],
                            op=mybir.AluOpType.subtract,
                        )
                    x16s[j] = (x16, xlo)

                def mm(m, j):
                    x16, xlo = x16s[j]
                    nc.tensor.matmul(
                        acc[m][:],
                        lhsT=oh[j][:, m * P : (m + 1) * P],
                        rhs=x16[:],
                        start=(j == 0),
                        stop=(j == JT - 1) and not split,
                        skip_group_check=True,
                    )
                    if split:
                        nc.tensor.matmul(
                            acc[m][:],
                            lhsT=oh[j][:, m * P : (m + 1) * P],
                            rhs=xlo[:],
                            start=False,
                            stop=(j == JT - 1),
                            skip_group_check=True,
                        )

                LAST = JT - CHUNKS[-1]     # final chunk runs m-major
                for j in range(JT):
                    t = oh_pool.tile([P, NR], F16, name=f"oh{j}", tag="oh")
                    nc.vector.tensor_scalar(
                        out=t[:],
                        in0=iota16[:],
                        scalar1=r_f[:, j : j + 1],
                        scalar2=None,
                        op0=mybir.AluOpType.is_equal,
                    )
                    oh.append(t)
                    load_x16(j)
                    if j == 0:
                        nc.vector.tensor_copy(oh_sum[:], t[:])
                    else:
                        nc.vector.tensor_tensor(
                            out=oh_sum[:], in0=oh_sum[:], in1=t[:],
                            op=mybir.AluOpType.add,
                        )
                    if j < LAST:
                        for m in range(MC):
                            mm(m, j)

                # counts + reciprocal + PE transpose to [128, 4] — emitted
                # late so the PE stream isn't blocked on the full oh_sum chain
                nc.tensor.matmul(
                    cnt[:], lhsT=ones_st[:], rhs=oh_sum[:],
                    start=True, stop=True, skip_group_check=True,
                )
                csb = out_pool.tile([1, NR], F32, tag="csb")
                nc.vector.tensor_scalar_max(csb[:], cnt[:], 1.0)
                recip = out_pool.tile([1, NR], F32, tag="recip")
                nc.vector.reciprocal(recip[:], csb[:])
                rt = out_pool.tile([P, MC], F32, tag="rt")
                for m in range(MC):
                    rp = psum_pool.tile([P, 1], F32, name=f"rp{m}", tag=f"rp{m % 2}")
                    nc.tensor.transpose(
                        rp[:], recip[:, m * P : (m + 1) * P], ident1[:]
                    )
                    nc.vector.tensor_copy(rt[:, m : m + 1], rp[:])

                osb = out_pool.tile([P, MC, C], F32, tag="osb")
                for m in range(MC):
                    for j in range(LAST, JT):
                        mm(m, j)
                    # --- Phase C (per m, overlaps later m's matmuls) ---
                    nc.vector.tensor_scalar(
                        out=osb[:, m, :],
                        in0=acc[m][:],
                        scalar1=rt[:, m : m + 1],
                        scalar2=None,
                        op0=mybir.AluOpType.mult,
                    )
                    nc.sync.dma_start(o_d[m * P : (m + 1) * P, :], osb[:, m, :])

            if repeats == 1:
                body()
            else:
                with tc.For_i(0, repeats, 1, hint_engines=(mybir.EngineType.PE,)):
                    body()

    nc.compile()
    return nc


def _get_nc(**cfg):
    cfg = {**DEFAULT_CFG, **cfg}
    key = tuple(sorted(cfg.items()))
    if key not in _CACHE:
        _CACHE[key] = _build(**cfg)
    return _CACHE[key]


def kernel(x, regions, max_n, _trace=False, _tmpdir=None, _cfg=None):
    x = np.asarray(x, dtype=np.float32)
    regions = np.asarray(regions)
    assert x.shape == (NCORES, T, C), x.shape
    assert regions.shape == (NCORES, T), regions.shape
    assert int(np.asarray(max_n)) == NR

    r32 = np.ascontiguousarray(regions.astype(np.int32))

    nc = _get_nc(**(_cfg or {}))
    in_maps = [
        {"x": np.ascontiguousarray(x[b]), "regions": r32[b]} for b in range(NCORES)
    ]
    res = run_bass_kernel_spmd(
        nc,
        in_maps,
        core_ids=list(range(NCORES)),
        trace=_trace,
        tmpdir=_tmpdir,
    )
    out = np.stack([res.results[b]["out"] for b in range(NCORES)], axis=0)
    if _trace:
        kernel._last_results = res
    return out


# revision 19
# speedup vs baseline: 1.1608x; 1.1477x over previous
"""Trainium2 Bass kernel for nn_LocalDownsample (segment mean-pool via one-hot matmul).

Contract: kernel(**inputs) takes FULL inputs (x [8,4096,512] f32,
regions [8,4096] i64, max_n=512), returns FULL output [8,512,512] f32.

Sharding: pure data parallel — batch b -> core b. Per core:
  out[n-1, :] = mean over tokens t with regions[t] == n of x[t, :]   (0 if empty)

Device algorithm per core (T=4096 tokens, C=512 channels, N=512 regions):
  tokens laid out as t = p*32 + j (p = SBUF partition, j = k-tile).
  Phase A (overlaps the ramped x DMA stream): build 32 one-hot tiles
    oh_j [128,512] fp16 = (iota == regions[p,j]) on DVE (all-fp16, 2x mode),
    accumulate oh_sum += oh_j on DVE; counts via one matmul
    cnt[1,512] = ones[128,1].T @ oh_sum; recip = 1/max(cnt,1), PE-transposed
    to rt [128,4] in the DMA shadow.
  Phase B: acc[m][128,512] fp32 PSUM += oh_j[:, mP:(m+1)P].T @ fp16(x)_j
    (+ residual matmul with fp16(x - fp16(x)) when split=True -> ~1e-7 rel err;
     without it ~2.5e-4). Final chunk runs m-major so acc banks close early.
  Phase C (per m, pipelined): osb_m = acc[m] * rt[:, m] on DVE, 256 KiB DMA out.
"""

import numpy as np

import concourse.bacc as bacc
import concourse.bass as bass  # noqa: F401
import concourse.mybir as mybir
import concourse.tile as tile
from concourse.bass_utils import run_bass_kernel_spmd

P = 128          # SBUF partitions
T = 4096         # tokens per batch
C = 512          # channels
NR = 512         # number of regions (max_n)
JT = T // P      # 32 k-tiles
MC = NR // P     # 4 output row chunks
NCORES = 8
CHUNKS = (1, 1, 2, 4, 8, 8, 8)   # k-tiles per x DMA chunk (ramped start)

F16 = mybir.dt.float16
F32 = mybir.dt.float32
I32 = mybir.dt.int32

DEFAULT_CFG = dict(split=True, repeats=1)

_CACHE = {}


def _build(split=True, repeats=1):
    assert sum(CHUNKS) == JT
    nc = bacc.Bacc(None, target_bir_lowering=False)
    x_d = nc.dram_tensor("x", [T, C], F32, kind="ExternalInput")
    r_d = nc.dram_tensor("regions", [T], I32, kind="ExternalInput")
    o_d = nc.dram_tensor("out", [NR, C], F32, kind="ExternalOutput")

    with tile.TileContext(nc) as tc:
        with (
            tc.tile_pool(name="const", bufs=1) as cpool,
            tc.tile_pool(name="xf", bufs=len(CHUNKS)) as xf_pool,
            tc.tile_pool(name="x16", bufs=10) as x16_pool,
            tc.tile_pool(name="oh", bufs=JT) as oh_pool,
            tc.tile_pool(name="eplg", bufs=1) as out_pool,
            tc.tile_pool(name="psum", bufs=1, space="PSUM") as psum_pool,
        ):
            # --- constants; regions ride the Activation HWDGE ring so the
            # SP ring belongs to the x stream from t=0 ---
            r_i = cpool.tile([P, JT], I32, tag="r_i")
            nc.scalar.dma_start(r_i[:], r_d.rearrange("(p j) -> p j", p=P))
            r_f = cpool.tile([P, JT], F32, tag="r_f")
            nc.vector.tensor_copy(r_f[:], r_i[:])

            iota16 = cpool.tile([P, NR], F16, tag="iota16")
            nc.gpsimd.iota(
                iota16[:], pattern=[[1, NR]], base=1, channel_multiplier=0,
                allow_small_or_imprecise_dtypes=True,  # 1..512 exact in fp16
            )

            ones_st = cpool.tile([P, 1], F32, tag="ones_st")
            nc.vector.memset(ones_st[:], 1.0)
            ident1 = cpool.tile([1, 1], F32, tag="ident1")
            nc.vector.memset(ident1[:], 1.0)

            def body():
                # x stream: queue all chunk DMAs up front (FIFO on the SP ring,
                # ramped sizes so the first matmuls can start early)
                xv = x_d.rearrange("(p j) c -> p j c", p=P)
                xf = []          # per j: (chunk_tile, index within chunk)
                j0 = 0
                for ci, csz in enumerate(CHUNKS):
                    t = xf_pool.tile([P, csz, C], F32, name=f"xfc{ci}", tag="xf")
                    nc.sync.dma_start(t[:], xv[:, j0 : j0 + csz, :])
                    for jj in range(csz):
                        xf.append((t, jj))
                    j0 += csz

                # one PSUM bank per accumulation group: start=True clears
                # has_written for the whole bank
                acc = [
                    psum_pool.tile([P, C], F32, name=f"acc{m}", tag=f"acc{m}")
                    for m in range(MC)
                ]
                cnt = psum_pool.tile([1, NR], F32, tag="cnt")

                # --- Phases A+B interleaved: per k-tile, DVE builds the
                # one-hot and running sum (and the fp16 residual of x when
                # split) while PE streams the main matmuls ---
                oh = []
                # fp32 so counts stay exact even if one region owned all
                # 4096 tokens (fp16 integers are only exact to 2048)
                oh_sum = out_pool.tile([P, NR], F32, tag="oh_sum")
                x16s = {}

                def load_x16(j):
                    xt, jj = xf[j]
                    x16 = x16_pool.tile([P, C], F16, name=f"x16_{j}", tag="x16")
                    if j < 2:
                        nc.vector.tensor_copy(x16[:], xt[:, jj, :])
                    else:
                        nc.scalar.copy(x16[:], xt[:, jj, :])
                    xlo = None
                    if split:
                        xlo = x16_pool.tile([P, C], F16, name=f"xlo_{j}", tag="xlo")
                        nc.vector.tensor_tensor(
                            out=xlo[:], in0=xt[:, jj, :], in1=x16[:],
                            op=mybir.AluOpType.subtract,
                        )
                    x16s[j] = (x16, xlo)

                def mm(m, j):
                    x16, xlo = x16s[j]
                    nc.tensor.matmul(
                        acc[m][:],
                        lhsT=oh[j][:, m * P : (m + 1) * P],
                        rhs=x16[:],
                        start=(j == 0),
                        stop=(j == JT - 1) and not split,
                        skip_group_check=True,
                    )
                    if split:
                        nc.tensor.matmul(
                            acc[m][:],
                            lhsT=oh[j][:, m * P : (m + 1) * P],
                            rhs=xlo[:],
                            start=False,
                            stop=(j == JT - 1),
                            skip_group_check=True,
                        )

                LAST = JT - CHUNKS[-1]     # final chunk runs m-major
                for j in range(JT):
                    t = oh_pool.tile([P, NR], F16, name=f"oh{j}", tag="oh")
                    nc.vector.tensor_scalar(
                        out=t[:],
                        in0=iota16[:],
                        scalar1=r_f[:, j : j + 1],
                        scalar2=None,
                        op0=mybir.AluOpType.is_equal,
                    )
                    oh.append(t)
                    load_x16(j)
                    if j == 0:
                        nc.vector.tensor_copy(oh_sum[:], t[:])
                    else:
                        nc.vector.tensor_tensor(
                            out=oh_sum[:], in0=oh_sum[:], in1=t[:],
                            op=mybir.AluOpType.add,
                        )
                    if j < LAST:
                        for m in range(MC):
                            mm(m, j)

                # counts + reciprocal + PE transpose to [128, 4] — emitted
                # late so the PE stream isn't blocked on the full oh_sum chain
                nc.tensor.matmul(
                    cnt[:], lhsT=ones_st[:], rhs=oh_sum[:],
                    start=True, stop=True, skip_group_check=True,
                )
                csb = out_pool.tile([1, NR], F32, tag="csb")
                nc.vector.tensor_scalar_max(csb[:], cnt[:], 1.0)
                recip = out_pool.tile([1, NR], F32, tag="recip")
                nc.vector.reciprocal(recip[:], csb[:])
                rt = out_pool.tile([P, MC], F32, tag="rt")
                for m in range(MC):
                    rp = psum_pool.tile([P, 1], F32, name=f"rp{m}", tag=f"rp{m % 2}")
                    nc.tensor.transpose(
                        rp[:], recip[:, m * P : (m + 1) * P], ident1[:]
                    )
                    nc.vector.tensor_copy(rt[:, m : m + 1], rp[:])

                osb = out_pool.tile([P, MC, C], F32, tag="osb")
                for m in range(MC):
                    for j in range(LAST, JT):
                        mm(m, j)
                    # --- Phase C (per m, overlaps later m's matmuls) ---
                    nc.vector.tensor_scalar(
                        out=osb[:, m, :],
                        in0=acc[m][:],
                        scalar1=rt[:, m : m + 1],
                        scalar2=None,
                        op0=mybir.AluOpType.mult,
                    )
                    nc.sync.dma_start(o_d[m * P : (m + 1) * P, :], osb[:, m, :])

            if repeats == 1:
                body()
            else:
                with tc.For_i(0, repeats, 1, hint_engines=(mybir.EngineType.PE,)):
                    body()

    nc.compile()
    return nc


def _get_nc(**cfg):
    cfg = {**DEFAULT_CFG, **cfg}
    key = tuple(sorted(cfg.items()))
    if key not in _CACHE:
        _CACHE[key] = _build(**cfg)
    return _CACHE[key]


def kernel(x, regions, max_n, _trace=False, _tmpdir=None, _cfg=None):
    x = np.asarray(x, dtype=np.float32)
    regions = np.asarray(regions)
    assert x.shape == (NCORES, T, C), x.shape
    assert regions.shape == (NCORES, T), regions.shape
    assert int(np.asarray(max_n)) == NR

    r32 = np.ascontiguousarray(regions.astype(np.int32))

    nc = _get_nc(**(_cfg or {}))
    in_maps = [
        {"x": np.ascontiguousarray(x[b]), "regions": r32[b]} for b in range(NCORES)
    ]
    res = run_bass_kernel_spmd(
        nc,
        in_maps,
        core_ids=list(range(NCORES)),
        trace=_trace,
        tmpdir=_tmpdir,
    )
    out = np.stack([res.results[b]["out"] for b in range(NCORES)], axis=0)
    if _trace:
        kernel._last_results = res
    return out
